# revision 23
# baseline (speedup 1.0000x reference)
"""Biaffine label attention kernel for 8 trn2 NeuronCores.

out[b, l, i, j] = (head[b] @ W_head.T)[i, l] + (dep[b] @ W_dep.T)[j, l] + bias[l]

with head/dep: [8, 512, 512] f32, label_W: [64, 1024], label_b: [64],
out: [8, 64, 512, 512] f32 (512 MB).

Sharding: data-parallel over batch; core b computes the contiguous 64 MB
slice out[b].  The kernel is output-write bound (~358 GB/s per core), so the
device program is organized to keep the output DMAs maximally efficient
(1 MB dma_starts with 8 KB contiguous DRAM runs) while TensorE / ScalarE /
VectorE generate tiles well under the DMA rate:

  - TensorE: tiny projections h = head@Wh^T (in a swizzled layout) and
    d = dep@Wd^T + b, then one K=1 ones-matmul per label to broadcast
    d[l, :] across 128 partitions into PSUM.
  - ScalarE: PSUM -> SBUF copy of the broadcast row block.
  - VectorE: 4x tensor_scalar_add per label (fp32 2x mode, SBUF->SBUF):
    out_tile[p, c*512 + j] = d[l, j] + h[l, 4p + c].
  - One 1 MB HWDGE DMA per label writes out[b, l] (partition p holds rows
    4p..4p+3 -> 8 KB contiguous runs in DRAM).
"""

import os
import sys
from contextlib import ExitStack

for _p in ("/opt/trn_rl_repo",):
    if os.path.isdir(_p) and _p not in sys.path:
        sys.path.insert(0, _p)

import numpy as np

import concourse.bass as bass
import concourse.bacc as bacc
import concourse.tile as tile
from concourse import mybir
from concourse.bass_utils import run_bass_kernel_spmd

B = 8
S = 512
D = 512
L = 64
KT = D // 128  # contraction tiles
C = S // 128   # i-rows packed per partition
F32 = mybir.dt.float32

_NC_CACHE = None


def _build_nc():
    nc = bacc.Bacc(
        "TRN2", target_bir_lowering=False, debug=False, num_devices=B
    )
    BF16 = mybir.dt.bfloat16
    headT = nc.declare_dram_parameter("headT", [128, KT * S], F32, isOutput=False)
    depTh = nc.declare_dram_parameter("depTh", [128, KT * S], BF16, isOutput=False)
    depTl = nc.declare_dram_parameter("depTl", [128, KT * S], BF16, isOutput=False)
    whT = nc.declare_dram_parameter("whT", [128, KT * L], F32, isOutput=False)
    wdTh = nc.declare_dram_parameter("wdTh", [128, KT * L], BF16, isOutput=False)
    wdTl = nc.declare_dram_parameter("wdTl", [128, KT * L], BF16, isOutput=False)
    biasv = nc.declare_dram_parameter("biasv", [L, 1], F32, isOutput=False)
    out = nc.declare_dram_parameter("out", [L, S, S], F32, isOutput=True)

    with tile.TileContext(nc) as tc, ExitStack() as ctx:
        const = ctx.enter_context(tc.tile_pool(name="const", bufs=1))
        psum_bc = ctx.enter_context(tc.tile_pool(name="psum_bc", bufs=6, space="PSUM"))
        psum_hd = ctx.enter_context(tc.tile_pool(name="psum_hd", bufs=1, space="PSUM"))
        out_pool = ctx.enter_context(tc.tile_pool(name="outp", bufs=6))

        # Input loads are split across the two HWDGE rings so both streams
        # land in parallel: d-path (bf16 hi/lo) on the scalar ring, h-path on
        # the sync ring (which afterwards is dedicated to output writes).
        wdh = const.tile([128, KT * L], BF16)
        nc.scalar.dma_start(wdh[:], wdTh[:, :])
        wdl = const.tile([128, KT * L], BF16)
        nc.scalar.dma_start(wdl[:], wdTl[:, :])
        bcol = const.tile([L, 1], F32)
        nc.scalar.dma_start(bcol[:], biasv[:, :])
        # dT in per-kt chunks so the d matmul starts before everything lands
        dTh, dTl = [], []
        for kt in range(KT):
            th = const.tile([128, S], BF16, tag=f"dTh{kt}")
            nc.scalar.dma_start(th[:], depTh[:, kt * S : (kt + 1) * S])
            dTh.append(th)
            tl = const.tile([128, S], BF16, tag=f"dTl{kt}")
            nc.scalar.dma_start(tl[:], depTl[:, kt * S : (kt + 1) * S])
            dTl.append(tl)
        wh = const.tile([128, KT * L], F32)
        nc.sync.dma_start(wh[:], whT[:, :])
        hT = const.tile([128, KT * S], F32)
        nc.sync.dma_start(hT[:], headT[:, :])

        ones2 = const.tile([2, 128], BF16)
        nc.vector.memset(ones2[:], 1.0)

        # d'[l, j] = sum_d dep[j, d] * W_dep[l, d] + b[l]   (l on partitions)
        # bf16x2: dep ~ dh + dl, wd ~ wh_ + wl_; accumulate the three large
        # cross terms in fp32 PSUM (the lo*lo term is ~2^-18 and dropped).
        dps = psum_hd.tile([L, S], F32)
        n_terms = 3 * KT
        ti = 0
        for kt in range(KT):
            for wa, db in ((wdh, dTh), (wdh, dTl), (wdl, dTh)):
                nc.tensor.matmul(
                    dps[:],
                    wa[:, kt * L : (kt + 1) * L],
                    db[kt][:],
                    start=(ti == 0),
                    stop=(ti == n_terms - 1),
                )
                ti += 1
        d_sb = const.tile([L, S], F32)
        nc.scalar.add(d_sb[:], dps[:], bcol[:])

        # Split d' into bf16 hi + lo so the per-label broadcast matmul runs at
        # full PE rate (fp32 matmul is ~8x slower); hi+lo recovers ~fp32
        # precision since PSUM accumulates in fp32.
        d_hi = const.tile([L, S], BF16)
        nc.vector.tensor_copy(d_hi[:], d_sb[:])
        d_hi32 = const.tile([L, S], F32)
        nc.scalar.copy(d_hi32[:], d_hi[:])
        d_lo = const.tile([L, S], BF16)
        nc.vector.tensor_sub(d_lo[:], d_sb[:], d_hi32[:])

        # Flatten [L, S] (l on partitions) -> rows of one [2, L*S] tile so the
        # broadcast rhs [2, S] can be sliced at partition base 0 for any l
        # (engine APs may only start at partition 0/32/64).
        d2 = const.tile([2, L * S], BF16)
        d2v = d2[:].rearrange("p (l j) -> p l j", l=L)
        nc.sync.dma_start(d2v[0:1, :, :], d_hi[:])
        nc.sync.dma_start(d2v[1:2, :, :], d_lo[:])

        # h_sw[p, c*L + l] = sum_d head[4p + c, d] * W_head[l, d]
        # headT arrives host-swizzled so lhsT slices are contiguous.
        hps = psum_hd.tile([128, C * L], F32)
        for c in range(C):
            for kt in range(KT):
                nc.tensor.matmul(
                    hps[:, c * L : (c + 1) * L],
                    hT[:, kt * S + c * 128 : kt * S + (c + 1) * 128],
                    wh[:, kt * L : (kt + 1) * L],
                    start=(kt == 0),
                    stop=(kt == KT - 1),
                )
        h_sw = const.tile([128, C * L], F32)
        nc.vector.tensor_copy(h_sw[:], hps[:])

        # out[l, 4p + c, j] = d'[l, j] + h_sw[p, c*L + l]
        # Two labels per SBUF tile -> one 2 MB output DMA per pair.
        out_r = out[:, :, :].rearrange("(lp m) (p c) j -> lp p m (c j)", m=2, c=C)
        for lp in range(L // 2):
            ot = out_pool.tile([128, 2 * C * S], F32)
            for m in range(2):
                l = 2 * lp + m
                bcp = psum_bc.tile([128, S], F32)
                nc.tensor.matmul(
                    bcp[:], ones2[:], d2v[:, l, :], start=True, stop=True
                )
                for c in range(C):
                    scalar = h_sw[:, c * L + l : c * L + l + 1]
                    dst = ot[:, (m * C + c) * S : (m * C + c + 1) * S]
                    if c < 2:
                        nc.vector.tensor_scalar_add(dst, bcp[:], scalar)
                    else:
                        nc.scalar.add(dst, bcp[:], scalar)
            nc.sync.dma_start(out_r[lp], ot[:])
    nc.compile()
    return nc


def _row_tile(a):
    """[D, F] -> [128, KT*F]: row d = kt*128 + p lands at [p, kt*F : (kt+1)*F]."""
    d, f = a.shape
    kt = d // 128
    return np.ascontiguousarray(
        a.reshape(kt, 128, f).transpose(1, 0, 2).reshape(128, kt * f)
    )


def _hi_lo(a):
    """f32 array -> (bf16 hi, bf16 lo) with a ~ hi + lo."""
    import ml_dtypes

    hi = a.astype(ml_dtypes.bfloat16)
    lo = (a - hi.astype(np.float32)).astype(ml_dtypes.bfloat16)
    return np.ascontiguousarray(hi), np.ascontiguousarray(lo)


def _prep_inputs(head, dep, label_W, label_b):
    head = np.asarray(head, dtype=np.float32)
    dep = np.asarray(dep, dtype=np.float32)
    label_W = np.asarray(label_W, dtype=np.float32)
    label_b = np.asarray(label_b, dtype=np.float32)

    wh = _row_tile(np.ascontiguousarray(label_W[:, :D].T))  # [128, KT*L]
    wdh, wdl = _hi_lo(_row_tile(np.ascontiguousarray(label_W[:, D:].T)))
    bias = np.ascontiguousarray(label_b.reshape(L, 1))

    in_maps = []
    for b in range(B):
        ht = head[b].T  # [D, S]
        # column swizzle: ht_sw[d, c*128 + m] = ht[d, 4m + c]
        ht_sw = ht.reshape(D, S // C, C).transpose(0, 2, 1).reshape(D, S)
        dth, dtl = _hi_lo(_row_tile(np.ascontiguousarray(dep[b].T)))
        in_maps.append(
            {
                "headT": _row_tile(ht_sw),
                "depTh": dth,
                "depTl": dtl,
                "whT": wh,
                "wdTh": wdh,
                "wdTl": wdl,
                "biasv": bias,
            }
        )
    return in_maps


def _run(head, dep, label_W, label_b, trace=False, **trace_kwargs):
    global _NC_CACHE
    if _NC_CACHE is None:
        _NC_CACHE = _build_nc()
    in_maps = _prep_inputs(head, dep, label_W, label_b)
    res = run_bass_kernel_spmd(
        _NC_CACHE, in_maps, list(range(B)), trace=trace, **trace_kwargs
    )
    out = np.stack([res.results[i]["out"] for i in range(B)])
    return out, res


def kernel(head, dep, label_W, label_b):
    out, _ = _run(head, dep, label_W, label_b, trace=False)
    return out


# revision 28
# speedup vs baseline: 1.0303x; 1.0303x over previous
"""Biaffine label attention kernel for 8 trn2 NeuronCores.

out[b, l, i, j] = (head[b] @ W_head.T)[i, l] + (dep[b] @ W_dep.T)[j, l] + bias[l]

with head/dep: [8, 512, 512] f32, label_W: [64, 1024], label_b: [64],
out: [8, 64, 512, 512] f32 (512 MB).

Sharding: data-parallel over batch; core b computes the contiguous 64 MB
slice out[b].  The kernel is output-write bound (~419 GB/s per core observed),
so the device program keeps the output DMAs maximally efficient (2 MB
dma_starts with 8 KB contiguous DRAM runs) while TensorE / ScalarE / VectorE
generate tiles well under the DMA rate:

  - TensorE: tiny projections h = head@Wh^T (in a swizzled layout) and
    d' = dep@Wd^T + b (fp32), then one K=2 ones-matmul per label over a
    bf16 hi/lo split of d' (full PE rate, fp32-accurate via PSUM fp32
    accumulation) to broadcast d'[l, :] across 128 partitions into PSUM.
  - VectorE + ScalarE: 4 per-partition-scalar adds per label, reading the
    broadcast PSUM tile: out_tile[p, c*512 + j] = d'[l, j] + h[l, 4p + c].
  - One 2 MB HWDGE DMA per label pair writes out[b, 2lp:2lp+2] (partition p
    holds rows 4p..4p+3 -> 8 KB contiguous runs in DRAM).

Measured on 8 axon trn2 cores: ~198.5 us HW exec, rel err ~1.7e-6
(output roofline at the observed ~419 GB/s steady DMA rate is ~160 us;
the gap is the serial d'/h prologue plus kernel preamble/drain).
"""

import os
import sys
from contextlib import ExitStack

for _p in ("/opt/trn_rl_repo",):
    if os.path.isdir(_p) and _p not in sys.path:
        sys.path.insert(0, _p)

import numpy as np

import concourse.bass as bass
import concourse.bacc as bacc
import concourse.tile as tile
from concourse import mybir
from concourse.bass_utils import run_bass_kernel_spmd

B = 8
S = 512
D = 512
L = 64
KT = D // 128  # contraction tiles
C = S // 128   # i-rows packed per partition
F32 = mybir.dt.float32

_NC_CACHE = None


def _build_nc():
    nc = bacc.Bacc(
        "TRN2", target_bir_lowering=False, debug=False, num_devices=B
    )
    BF16 = mybir.dt.bfloat16
    headT = nc.declare_dram_parameter("headT", [128, KT * S], F32, isOutput=False)
    depT = nc.declare_dram_parameter("depT", [128, KT * S], F32, isOutput=False)
    whT = nc.declare_dram_parameter("whT", [128, KT * L], F32, isOutput=False)
    wdT = nc.declare_dram_parameter("wdT", [128, KT * L], F32, isOutput=False)
    biasv = nc.declare_dram_parameter("biasv", [L, 1], F32, isOutput=False)
    out = nc.declare_dram_parameter("out", [L, S, S], F32, isOutput=True)

    with tile.TileContext(nc) as tc, ExitStack() as ctx:
        const = ctx.enter_context(tc.tile_pool(name="const", bufs=1))
        psum_bc = ctx.enter_context(tc.tile_pool(name="psum_bc", bufs=6, space="PSUM"))
        psum_hd = ctx.enter_context(tc.tile_pool(name="psum_hd", bufs=1, space="PSUM"))
        out_pool = ctx.enter_context(tc.tile_pool(name="outp", bufs=6))

        # All loads share the sync HWDGE ring with the output writes — using
        # the scalar ring for inputs measurably slowed the steady-state
        # output stream (~400 vs ~419 GB/s), so everything stays on one ring.
        # d-path inputs first: the whole kernel is gated on d' being ready.
        wd = const.tile([128, KT * L], F32)
        nc.sync.dma_start(wd[:], wdT[:, :])
        bcol = const.tile([L, 1], F32)
        nc.sync.dma_start(bcol[:], biasv[:, :])
        # dT in per-kt chunks so the d matmul starts before the full 1 MB lands
        dT = []
        for kt in range(KT):
            t = const.tile([128, S], F32, tag=f"dT{kt}")
            nc.sync.dma_start(t[:], depT[:, kt * S : (kt + 1) * S])
            dT.append(t)
        wh = const.tile([128, KT * L], F32)
        nc.sync.dma_start(wh[:], whT[:, :])
        hT = const.tile([128, KT * S], F32)
        nc.sync.dma_start(hT[:], headT[:, :])

        ones2 = const.tile([2, 128], BF16)
        nc.vector.memset(ones2[:], 1.0)

        # d'[l, j] = sum_d dep[j, d] * W_dep[l, d] + b[l]   (l on partitions)
        dps = psum_hd.tile([L, S], F32)
        for kt in range(KT):
            nc.tensor.matmul(
                dps[:],
                wd[:, kt * L : (kt + 1) * L],
                dT[kt][:],
                start=(kt == 0),
                stop=(kt == KT - 1),
            )
        d_sb = const.tile([L, S], F32)
        nc.scalar.add(d_sb[:], dps[:], bcol[:])

        # Split d' into bf16 hi + lo so the per-label broadcast matmul runs at
        # full PE rate (fp32 matmul is ~8x slower); hi+lo recovers ~fp32
        # precision since PSUM accumulates in fp32.
        d_hi = const.tile([L, S], BF16)
        nc.vector.tensor_copy(d_hi[:], d_sb[:])
        d_hi32 = const.tile([L, S], F32)
        nc.scalar.copy(d_hi32[:], d_hi[:])
        d_lo = const.tile([L, S], BF16)
        nc.vector.tensor_sub(d_lo[:], d_sb[:], d_hi32[:])

        # Flatten [L, S] (l on partitions) -> rows of one [2, L*S] tile so the
        # broadcast rhs [2, S] can be sliced at partition base 0 for any l
        # (engine APs may only start at partition 0/32/64).
        d2 = const.tile([2, L * S], BF16)
        d2v = d2[:].rearrange("p (l j) -> p l j", l=L)
        nc.sync.dma_start(d2v[0:1, :, :], d_hi[:])
        nc.sync.dma_start(d2v[1:2, :, :], d_lo[:])

        # h_sw[p, c*L + l] = sum_d head[4p + c, d] * W_head[l, d]
        # headT arrives host-swizzled so lhsT slices are contiguous.
        hps = psum_hd.tile([128, C * L], F32)
        for c in range(C):
            for kt in range(KT):
                nc.tensor.matmul(
                    hps[:, c * L : (c + 1) * L],
                    hT[:, kt * S + c * 128 : kt * S + (c + 1) * 128],
                    wh[:, kt * L : (kt + 1) * L],
                    start=(kt == 0),
                    stop=(kt == KT - 1),
                )
        h_sw = const.tile([128, C * L], F32)
        nc.scalar.copy(h_sw[:], hps[:])

        # out[l, 4p + c, j] = d'[l, j] + h_sw[p, c*L + l]
        # Two labels per SBUF tile -> one 2 MB output DMA per pair.
        out_r = out[:, :, :].rearrange("(lp m) (p c) j -> lp p m (c j)", m=2, c=C)
        for lp in range(L // 2):
            ot = out_pool.tile([128, 2 * C * S], F32)
            for m in range(2):
                l = 2 * lp + m
                bcp = psum_bc.tile([128, S], F32)
                nc.tensor.matmul(
                    bcp[:], ones2[:], d2v[:, l, :], start=True, stop=True
                )
                for c in range(C):
                    scalar = h_sw[:, c * L + l : c * L + l + 1]
                    dst = ot[:, (m * C + c) * S : (m * C + c + 1) * S]
                    if c < 2:
                        nc.vector.tensor_scalar_add(dst, bcp[:], scalar)
                    else:
                        nc.scalar.add(dst, bcp[:], scalar)
            nc.sync.dma_start(out_r[lp], ot[:])
    nc.compile()
    return nc


def _row_tile(a):
    """[D, F] -> [128, KT*F]: row d = kt*128 + p lands at [p, kt*F : (kt+1)*F]."""
    d, f = a.shape
    kt = d // 128
    return np.ascontiguousarray(
        a.reshape(kt, 128, f).transpose(1, 0, 2).reshape(128, kt * f)
    )


def _prep_inputs(head, dep, label_W, label_b):
    head = np.asarray(head, dtype=np.float32)
    dep = np.asarray(dep, dtype=np.float32)
    label_W = np.asarray(label_W, dtype=np.float32)
    label_b = np.asarray(label_b, dtype=np.float32)

    wh = _row_tile(np.ascontiguousarray(label_W[:, :D].T))  # [128, KT*L]
    wd = _row_tile(np.ascontiguousarray(label_W[:, D:].T))
    bias = np.ascontiguousarray(label_b.reshape(L, 1))

    in_maps = []
    for b in range(B):
        ht = head[b].T  # [D, S]
        # column swizzle: ht_sw[d, c*128 + m] = ht[d, 4m + c]
        ht_sw = ht.reshape(D, S // C, C).transpose(0, 2, 1).reshape(D, S)
        in_maps.append(
            {
                "headT": _row_tile(ht_sw),
                "depT": _row_tile(np.ascontiguousarray(dep[b].T)),
                "whT": wh,
                "wdT": wd,
                "biasv": bias,
            }
        )
    return in_maps


def _run(head, dep, label_W, label_b, trace=False, **trace_kwargs):
    global _NC_CACHE
    if _NC_CACHE is None:
        _NC_CACHE = _build_nc()
    in_maps = _prep_inputs(head, dep, label_W, label_b)
    res = run_bass_kernel_spmd(
        _NC_CACHE, in_maps, list(range(B)), trace=trace, **trace_kwargs
    )
    out = np.stack([res.results[i]["out"] for i in range(B)])
    return out, res


def kernel(head, dep, label_W, label_b):
    out, _ = _run(head, dep, label_W, label_b, trace=False)
    return out


# revision 32
# speedup vs baseline: 1.1663x; 1.1320x over previous
"""Biaffine label attention kernel for 8 trn2 NeuronCores.

out[b, l, i, j] = (head[b] @ W_head.T)[i, l] + (dep[b] @ W_dep.T)[j, l] + bias[l]

with head/dep: [8, 512, 512] f32, label_W: [64, 1024], label_b: [64],
out: [8, 64, 512, 512] f32 (512 MB).

Sharding: data-parallel over batch; core b computes the contiguous 64 MB
slice out[b].  The kernel is output-write bound (~419 GB/s per core observed),
so the device program keeps the output DMAs maximally efficient (2 MB
dma_starts with 8 KB contiguous DRAM runs) while TensorE / ScalarE / VectorE
generate tiles well under the DMA rate:

  - TensorE: tiny projections h = head@Wh^T (in a swizzled layout) and
    d' = dep@Wd^T + b (fp32), then one K=2 ones-matmul per label over a
    bf16 hi/lo split of d' (full PE rate, fp32-accurate via PSUM fp32
    accumulation) to broadcast d'[l, :] across 128 partitions into PSUM.
  - VectorE + ScalarE: 4 per-partition-scalar adds per label, reading the
    broadcast PSUM tile: out_tile[p, c*512 + j] = d'[l, j] + h[l, 4p + c].
  - One 2 MB HWDGE DMA per label pair writes out[b, 2lp:2lp+2] (partition p
    holds rows 4p..4p+3 -> 8 KB contiguous runs in DRAM).

Measured on 8 axon trn2 cores: ~198.5 us HW exec, rel err ~1.7e-6
(output roofline at the observed ~419 GB/s steady DMA rate is ~160 us;
the gap is the serial d'/h prologue plus kernel preamble/drain).
"""

import os
import sys
from contextlib import ExitStack

for _p in ("/opt/trn_rl_repo",):
    if os.path.isdir(_p) and _p not in sys.path:
        sys.path.insert(0, _p)

import numpy as np

import concourse.bass as bass
import concourse.bacc as bacc
import concourse.tile as tile
from concourse import mybir
from concourse.bass_utils import run_bass_kernel_spmd

B = 8
S = 512
D = 512
L = 64
KT = D // 128  # contraction tiles
C = S // 128   # i-rows packed per partition
F32 = mybir.dt.float32

_NC_CACHE = None


def _build_nc():
    nc = bacc.Bacc(
        "TRN2", target_bir_lowering=False, debug=False, num_devices=B
    )
    BF16 = mybir.dt.bfloat16
    headT = nc.declare_dram_parameter("headT", [128, KT * S], F32, isOutput=False)
    depTh = nc.declare_dram_parameter("depTh", [128, KT * S], BF16, isOutput=False)
    depTl = nc.declare_dram_parameter("depTl", [128, KT * S], BF16, isOutput=False)
    whT = nc.declare_dram_parameter("whT", [128, KT * L], F32, isOutput=False)
    wdTh = nc.declare_dram_parameter("wdTh", [128, KT * L], BF16, isOutput=False)
    wdTl = nc.declare_dram_parameter("wdTl", [128, KT * L], BF16, isOutput=False)
    biasv = nc.declare_dram_parameter("biasv", [L, 1], F32, isOutput=False)
    out = nc.declare_dram_parameter("out", [L, S, S], F32, isOutput=True)

    with tile.TileContext(nc) as tc, ExitStack() as ctx:
        const = ctx.enter_context(tc.tile_pool(name="const", bufs=1))
        psum_bc = ctx.enter_context(tc.tile_pool(name="psum_bc", bufs=6, space="PSUM"))
        psum_hd = ctx.enter_context(tc.tile_pool(name="psum_hd", bufs=1, space="PSUM"))
        out_pool = ctx.enter_context(tc.tile_pool(name="outp", bufs=6))

        # All loads share the sync HWDGE ring with the output writes — using
        # the scalar ring for inputs measurably slowed the steady-state
        # output stream (~400 vs ~419 GB/s), so everything stays on one ring.
        # d-path inputs first: the whole kernel is gated on d' being ready.
        wdh = const.tile([128, KT * L], BF16)
        nc.sync.dma_start(wdh[:], wdTh[:, :])
        wdl = const.tile([128, KT * L], BF16)
        nc.sync.dma_start(wdl[:], wdTl[:, :])
        bcol = const.tile([L, 1], F32)
        nc.sync.dma_start(bcol[:], biasv[:, :])
        # dep hi/lo in per-kt chunks so the d matmul starts before everything
        # lands; hi/lo interleaved since each kt needs both.
        dTh, dTl = [], []
        for kt in range(KT):
            th = const.tile([128, S], BF16, tag=f"dTh{kt}")
            nc.sync.dma_start(th[:], depTh[:, kt * S : (kt + 1) * S])
            dTh.append(th)
            tl = const.tile([128, S], BF16, tag=f"dTl{kt}")
            nc.sync.dma_start(tl[:], depTl[:, kt * S : (kt + 1) * S])
            dTl.append(tl)
        wh = const.tile([128, KT * L], F32)
        nc.sync.dma_start(wh[:], whT[:, :])
        hT = const.tile([128, KT * S], F32)
        nc.sync.dma_start(hT[:], headT[:, :])

        ones2 = const.tile([2, 128], BF16)
        nc.vector.memset(ones2[:], 1.0)

        # d'[l, j] = sum_d dep[j, d] * W_dep[l, d] + b[l]   (l on partitions)
        # dep ~ dh + dl, wd ~ wh_ + wl_ in bf16; the three large cross terms
        # accumulate in fp32 PSUM at full PE rate (fp32 matmul is ~8x slower;
        # the dropped lo*lo term is ~2^-18 relative).
        dps = psum_hd.tile([L, S], F32)
        n_terms = 3 * KT
        ti = 0
        for kt in range(KT):
            for wa, db in ((wdh, dTh), (wdh, dTl), (wdl, dTh)):
                nc.tensor.matmul(
                    dps[:],
                    wa[:, kt * L : (kt + 1) * L],
                    db[kt][:],
                    start=(ti == 0),
                    stop=(ti == n_terms - 1),
                )
                ti += 1
        d_sb = const.tile([L, S], F32)
        nc.scalar.add(d_sb[:], dps[:], bcol[:])

        # Split d' into bf16 hi + lo so the per-label broadcast matmul runs at
        # full PE rate (fp32 matmul is ~8x slower); hi+lo recovers ~fp32
        # precision since PSUM accumulates in fp32.
        d_hi = const.tile([L, S], BF16)
        nc.vector.tensor_copy(d_hi[:], d_sb[:])
        d_hi32 = const.tile([L, S], F32)
        nc.scalar.copy(d_hi32[:], d_hi[:])
        d_lo = const.tile([L, S], BF16)
        nc.vector.tensor_sub(d_lo[:], d_sb[:], d_hi32[:])

        # Flatten [L, S] (l on partitions) -> rows of one [2, L*S] tile so the
        # broadcast rhs [2, S] can be sliced at partition base 0 for any l
        # (engine APs may only start at partition 0/32/64).
        d2 = const.tile([2, L * S], BF16)
        d2v = d2[:].rearrange("p (l j) -> p l j", l=L)
        nc.sync.dma_start(d2v[0:1, :, :], d_hi[:])
        nc.sync.dma_start(d2v[1:2, :, :], d_lo[:])

        # h_sw[p, c*L + l] = sum_d head[4p + c, d] * W_head[l, d]
        # headT arrives host-swizzled so lhsT slices are contiguous.
        hps = psum_hd.tile([128, C * L], F32)
        for c in range(C):
            for kt in range(KT):
                nc.tensor.matmul(
                    hps[:, c * L : (c + 1) * L],
                    hT[:, kt * S + c * 128 : kt * S + (c + 1) * 128],
                    wh[:, kt * L : (kt + 1) * L],
                    start=(kt == 0),
                    stop=(kt == KT - 1),
                )
        h_sw = const.tile([128, C * L], F32)
        nc.scalar.copy(h_sw[:], hps[:])

        # out[l, 4p + c, j] = d'[l, j] + h_sw[p, c*L + l]
        # The first few labels ship as single 1 MB DMAs so output bytes start
        # flowing as early as possible; the rest as 2 MB label-pair DMAs.
        N_WARM = 4
        out_r1 = out[:, :, :].rearrange("l (p c) j -> l p (c j)", c=C)
        out_r = out[:, :, :].rearrange("(lp m) (p c) j -> lp p m (c j)", m=2, c=C)

        def emit_label(l, ot, fbase):
            """Broadcast d'[l] and add h columns into ot[:, fbase:fbase+C*S]."""
            bcp = psum_bc.tile([128, S], F32)
            nc.tensor.matmul(bcp[:], ones2[:], d2v[:, l, :], start=True, stop=True)
            for c in range(C):
                scalar = h_sw[:, c * L + l : c * L + l + 1]
                dst = ot[:, fbase + c * S : fbase + (c + 1) * S]
                if c < 2:
                    nc.vector.tensor_scalar_add(dst, bcp[:], scalar)
                else:
                    nc.scalar.add(dst, bcp[:], scalar)

        warm_pool = ctx.enter_context(tc.tile_pool(name="warm", bufs=2))
        for l in range(N_WARM):
            ot = warm_pool.tile([128, C * S], F32)
            emit_label(l, ot, 0)
            nc.sync.dma_start(out_r1[l], ot[:])
        for lp in range(N_WARM // 2, L // 2):
            ot = out_pool.tile([128, 2 * C * S], F32)
            for m in range(2):
                emit_label(2 * lp + m, ot, m * C * S)
            nc.sync.dma_start(out_r[lp], ot[:])
    nc.compile()
    return nc


def _row_tile(a):
    """[D, F] -> [128, KT*F]: row d = kt*128 + p lands at [p, kt*F : (kt+1)*F]."""
    d, f = a.shape
    kt = d // 128
    return np.ascontiguousarray(
        a.reshape(kt, 128, f).transpose(1, 0, 2).reshape(128, kt * f)
    )


def _hi_lo(a):
    """f32 array -> (bf16 hi, bf16 lo) with a ~ hi + lo."""
    import ml_dtypes

    hi = a.astype(ml_dtypes.bfloat16)
    lo = (a - hi.astype(np.float32)).astype(ml_dtypes.bfloat16)
    return np.ascontiguousarray(hi), np.ascontiguousarray(lo)


def _prep_inputs(head, dep, label_W, label_b):
    head = np.asarray(head, dtype=np.float32)
    dep = np.asarray(dep, dtype=np.float32)
    label_W = np.asarray(label_W, dtype=np.float32)
    label_b = np.asarray(label_b, dtype=np.float32)

    wh = _row_tile(np.ascontiguousarray(label_W[:, :D].T))  # [128, KT*L]
    wdh, wdl = _hi_lo(_row_tile(np.ascontiguousarray(label_W[:, D:].T)))
    bias = np.ascontiguousarray(label_b.reshape(L, 1))

    in_maps = []
    for b in range(B):
        ht = head[b].T  # [D, S]
        # column swizzle: ht_sw[d, c*128 + m] = ht[d, 4m + c]
        ht_sw = ht.reshape(D, S // C, C).transpose(0, 2, 1).reshape(D, S)
        dth, dtl = _hi_lo(_row_tile(np.ascontiguousarray(dep[b].T)))
        in_maps.append(
            {
                "headT": _row_tile(ht_sw),
                "depTh": dth,
                "depTl": dtl,
                "whT": wh,
                "wdTh": wdh,
                "wdTl": wdl,
                "biasv": bias,
            }
        )
    return in_maps


def _run(head, dep, label_W, label_b, trace=False, **trace_kwargs):
    global _NC_CACHE
    if _NC_CACHE is None:
        _NC_CACHE = _build_nc()
    in_maps = _prep_inputs(head, dep, label_W, label_b)
    res = run_bass_kernel_spmd(
        _NC_CACHE, in_maps, list(range(B)), trace=trace, **trace_kwargs
    )
    out = np.stack([res.results[i]["out"] for i in range(B)])
    return out, res


def kernel(head, dep, label_W, label_b):
    out, _ = _run(head, dep, label_W, label_b, trace=False)
    return out


# revision 37
# speedup vs baseline: 1.1697x; 1.0029x over previous
"""Biaffine label attention kernel for 8 trn2 NeuronCores.

out[b, l, i, j] = (head[b] @ W_head.T)[i, l] + (dep[b] @ W_dep.T)[j, l] + bias[l]

with head/dep: [8, 512, 512] f32, label_W: [64, 1024], label_b: [64],
out: [8, 64, 512, 512] f32 (512 MB).

Sharding: data-parallel over batch; core b computes the contiguous 64 MB
slice out[b].  The kernel is output-write bound (~419 GB/s per core observed),
so the device program keeps the output DMAs maximally efficient (2 MB
dma_starts with 8 KB contiguous DRAM runs) while TensorE / ScalarE / VectorE
generate tiles well under the DMA rate:

  - TensorE: tiny projections h = head@Wh^T (in a swizzled layout) and
    d' = dep@Wd^T + b (fp32), then one K=2 ones-matmul per label over a
    bf16 hi/lo split of d' (full PE rate, fp32-accurate via PSUM fp32
    accumulation) to broadcast d'[l, :] across 128 partitions into PSUM.
  - VectorE + ScalarE: 4 per-partition-scalar adds per label, reading the
    broadcast PSUM tile: out_tile[p, c*512 + j] = d'[l, j] + h[l, 4p + c].
  - One 2 MB HWDGE DMA per label pair writes out[b, 2lp:2lp+2] (partition p
    holds rows 4p..4p+3 -> 8 KB contiguous runs in DRAM).

Measured on 8 axon trn2 cores: ~198.5 us HW exec, rel err ~1.7e-6
(output roofline at the observed ~419 GB/s steady DMA rate is ~160 us;
the gap is the serial d'/h prologue plus kernel preamble/drain).
"""

import os
import sys
from contextlib import ExitStack

for _p in ("/opt/trn_rl_repo",):
    if os.path.isdir(_p) and _p not in sys.path:
        sys.path.insert(0, _p)

import numpy as np

import concourse.bass as bass
import concourse.bacc as bacc
import concourse.masks as masks
import concourse.tile as tile
from concourse import mybir
from concourse.bass_utils import run_bass_kernel_spmd

B = 8
S = 512
D = 512
L = 64
KT = D // 128  # contraction tiles
C = S // 128   # i-rows packed per partition
F32 = mybir.dt.float32

_NC_CACHE = None


def _build_nc():
    nc = bacc.Bacc(
        "TRN2", target_bir_lowering=False, debug=False, num_devices=B
    )
    BF16 = mybir.dt.bfloat16
    headTh = nc.declare_dram_parameter("headTh", [128, KT * S], BF16, isOutput=False)
    headTl = nc.declare_dram_parameter("headTl", [128, KT * S], BF16, isOutput=False)
    depTh = nc.declare_dram_parameter("depTh", [128, KT * S], BF16, isOutput=False)
    depTl = nc.declare_dram_parameter("depTl", [128, KT * S], BF16, isOutput=False)
    whTh = nc.declare_dram_parameter("whTh", [128, KT * L], BF16, isOutput=False)
    whTl = nc.declare_dram_parameter("whTl", [128, KT * L], BF16, isOutput=False)
    wdTh = nc.declare_dram_parameter("wdTh", [128, KT * L], BF16, isOutput=False)
    wdTl = nc.declare_dram_parameter("wdTl", [128, KT * L], BF16, isOutput=False)
    biasv = nc.declare_dram_parameter("biasv", [L, 1], F32, isOutput=False)
    out = nc.declare_dram_parameter("out", [L, S, S], F32, isOutput=True)

    with tile.TileContext(nc) as tc, ExitStack() as ctx:
        const = ctx.enter_context(tc.tile_pool(name="const", bufs=1))
        psum_bc = ctx.enter_context(tc.tile_pool(name="psum_bc", bufs=5, space="PSUM"))
        psum_hd = ctx.enter_context(tc.tile_pool(name="psum_hd", bufs=1, space="PSUM"))
        out_pool = ctx.enter_context(tc.tile_pool(name="outp", bufs=6))

        # All loads share the sync HWDGE ring with the output writes — using
        # the scalar ring for inputs measurably slowed the steady-state
        # output stream (~400 vs ~419 GB/s), so everything stays on one ring.
        # d-path inputs first: the whole kernel is gated on d' being ready.
        wdh = const.tile([128, KT * L], BF16)
        nc.sync.dma_start(wdh[:], wdTh[:, :])
        wdl = const.tile([128, KT * L], BF16)
        nc.sync.dma_start(wdl[:], wdTl[:, :])
        bcol = const.tile([L, 1], F32)
        nc.sync.dma_start(bcol[:], biasv[:, :])
        # dep hi/lo in per-kt chunks so the d matmul starts before everything
        # lands; hi/lo interleaved since each kt needs both.
        dTh, dTl = [], []
        for kt in range(KT):
            th = const.tile([128, S], BF16, tag=f"dTh{kt}")
            nc.sync.dma_start(th[:], depTh[:, kt * S : (kt + 1) * S])
            dTh.append(th)
            tl = const.tile([128, S], BF16, tag=f"dTl{kt}")
            nc.sync.dma_start(tl[:], depTl[:, kt * S : (kt + 1) * S])
            dTl.append(tl)
        whh = const.tile([128, KT * L], BF16)
        nc.sync.dma_start(whh[:], whTh[:, :])
        whl = const.tile([128, KT * L], BF16)
        nc.sync.dma_start(whl[:], whTl[:, :])
        hTh = const.tile([128, KT * S], BF16)
        nc.sync.dma_start(hTh[:], headTh[:, :])
        hTl = const.tile([128, KT * S], BF16)
        nc.sync.dma_start(hTl[:], headTl[:, :])

        ones2 = const.tile([2, 128], BF16)
        nc.vector.memset(ones2[:], 1.0)
        ident = const.tile([L, L], F32)
        masks.make_identity(nc, ident[:])

        # d'[l, j] = sum_d dep[j, d] * W_dep[l, d] + b[l]   (l on partitions)
        # dep ~ dh + dl, wd ~ wh_ + wl_ in bf16; the three large cross terms
        # accumulate in fp32 PSUM at full PE rate (fp32 matmul is ~8x slower;
        # the dropped lo*lo term is ~2^-18 relative).
        dps = psum_hd.tile([L, S], F32)
        n_terms = 3 * KT
        ti = 0
        for kt in range(KT):
            for wa, db in ((wdh, dTh), (wdh, dTl), (wdl, dTh)):
                nc.tensor.matmul(
                    dps[:],
                    wa[:, kt * L : (kt + 1) * L],
                    db[kt][:],
                    start=(ti == 0),
                    stop=(ti == n_terms - 1),
                )
                ti += 1
        d_sb = const.tile([L, S], F32)
        nc.scalar.add(d_sb[:], dps[:], bcol[:])

        # Split d' into bf16 hi + lo so the per-label broadcast matmul runs at
        # full PE rate (fp32 matmul is ~8x slower); hi+lo recovers ~fp32
        # precision since PSUM accumulates in fp32.
        d_hi = const.tile([L, S], BF16)
        nc.vector.tensor_copy(d_hi[:], d_sb[:])
        d_hi32 = const.tile([L, S], F32)
        nc.scalar.copy(d_hi32[:], d_hi[:])
        d_lo = const.tile([L, S], BF16)
        nc.vector.tensor_sub(d_lo[:], d_sb[:], d_hi32[:])

        # Flatten [L, S] (l on partitions) -> rows of one [2, L*S] tile so the
        # broadcast rhs [2, S] can be sliced at partition base 0 for any l
        # (engine APs may only start at partition 0/32/64).
        d2 = const.tile([2, L * S], BF16)
        d2v = d2[:].rearrange("p (l j) -> p l j", l=L)
        nc.sync.dma_start(d2v[0:1, :, :], d_hi[:])
        nc.sync.dma_start(d2v[1:2, :, :], d_lo[:])

        # h[l, i] = sum_d head[i, d] * W_head[l, d], as bf16 hi/lo cross terms
        # (full PE rate; fp32 N=64 matmuls measured 2 HW passes = ~13 us and
        # gated the whole pipeline).  Result lands [l, i]; four strided PE
        # transposes then produce the swizzled [i, l] layout the adds need:
        # h_sw[p, c*L + l] = h[l, 4p + c].
        hps_li = psum_hd.tile([L, S], F32)
        ti = 0
        for kt in range(KT):
            for wa, ha in ((whh, hTh), (whh, hTl), (whl, hTh)):
                nc.tensor.matmul(
                    hps_li[:],
                    wa[:, kt * L : (kt + 1) * L],
                    ha[:, kt * S : (kt + 1) * S],
                    start=(ti == 0),
                    stop=(ti == 3 * KT - 1),
                )
                ti += 1
        h_li = const.tile([L, S], F32)
        nc.scalar.copy(h_li[:], hps_li[:])
        h_li_str = h_li[:].rearrange("l (m c) -> l c m", c=C)
        hps_sw = psum_hd.tile([128, C * L], F32)
        for c in range(C):
            nc.tensor.transpose(
                hps_sw[:, c * L : (c + 1) * L], h_li_str[:, c, :], ident[:]
            )
        h_sw = const.tile([128, C * L], F32)
        nc.scalar.copy(h_sw[:], hps_sw[:])

        # out[l, 4p + c, j] = d'[l, j] + h_sw[p, c*L + l]
        # The first few labels ship as single 1 MB DMAs so output bytes start
        # flowing as early as possible; the rest as 2 MB label-pair DMAs.
        N_WARM = 4
        out_r1 = out[:, :, :].rearrange("l (p c) j -> l p (c j)", c=C)
        out_r = out[:, :, :].rearrange("(lp m) (p c) j -> lp p m (c j)", m=2, c=C)

        def emit_label(l, ot, fbase):
            """Broadcast d'[l] and add h columns into ot[:, fbase:fbase+C*S]."""
            bcp = psum_bc.tile([128, S], F32)
            nc.tensor.matmul(bcp[:], ones2[:], d2v[:, l, :], start=True, stop=True)
            for c in range(C):
                scalar = h_sw[:, c * L + l : c * L + l + 1]
                dst = ot[:, fbase + c * S : fbase + (c + 1) * S]
                if c < 2:
                    nc.vector.tensor_scalar_add(dst, bcp[:], scalar)
                else:
                    nc.scalar.add(dst, bcp[:], scalar)

        warm_pool = ctx.enter_context(tc.tile_pool(name="warm", bufs=2))
        for l in range(N_WARM):
            ot = warm_pool.tile([128, C * S], F32)
            emit_label(l, ot, 0)
            nc.sync.dma_start(out_r1[l], ot[:])
        for lp in range(N_WARM // 2, L // 2):
            ot = out_pool.tile([128, 2 * C * S], F32)
            for m in range(2):
                emit_label(2 * lp + m, ot, m * C * S)
            nc.sync.dma_start(out_r[lp], ot[:])
    nc.compile()
    return nc


def _row_tile(a):
    """[D, F] -> [128, KT*F]: row d = kt*128 + p lands at [p, kt*F : (kt+1)*F]."""
    d, f = a.shape
    kt = d // 128
    return np.ascontiguousarray(
        a.reshape(kt, 128, f).transpose(1, 0, 2).reshape(128, kt * f)
    )


def _hi_lo(a):
    """f32 array -> (bf16 hi, bf16 lo) with a ~ hi + lo."""
    import ml_dtypes

    hi = a.astype(ml_dtypes.bfloat16)
    lo = (a - hi.astype(np.float32)).astype(ml_dtypes.bfloat16)
    return np.ascontiguousarray(hi), np.ascontiguousarray(lo)


def _prep_inputs(head, dep, label_W, label_b):
    head = np.asarray(head, dtype=np.float32)
    dep = np.asarray(dep, dtype=np.float32)
    label_W = np.asarray(label_W, dtype=np.float32)
    label_b = np.asarray(label_b, dtype=np.float32)

    whh, whl = _hi_lo(_row_tile(np.ascontiguousarray(label_W[:, :D].T)))
    wdh, wdl = _hi_lo(_row_tile(np.ascontiguousarray(label_W[:, D:].T)))
    bias = np.ascontiguousarray(label_b.reshape(L, 1))

    in_maps = []
    for b in range(B):
        hth, htl = _hi_lo(_row_tile(np.ascontiguousarray(head[b].T)))
        dth, dtl = _hi_lo(_row_tile(np.ascontiguousarray(dep[b].T)))
        in_maps.append(
            {
                "headTh": hth,
                "headTl": htl,
                "depTh": dth,
                "depTl": dtl,
                "whTh": whh,
                "whTl": whl,
                "wdTh": wdh,
                "wdTl": wdl,
                "biasv": bias,
            }
        )
    return in_maps


def _run(head, dep, label_W, label_b, trace=False, **trace_kwargs):
    global _NC_CACHE
    if _NC_CACHE is None:
        _NC_CACHE = _build_nc()
    in_maps = _prep_inputs(head, dep, label_W, label_b)
    res = run_bass_kernel_spmd(
        _NC_CACHE, in_maps, list(range(B)), trace=trace, **trace_kwargs
    )
    out = np.stack([res.results[i]["out"] for i in range(B)])
    return out, res


def kernel(head, dep, label_W, label_b):
    out, _ = _run(head, dep, label_W, label_b, trace=False)
    return out


# revision 42
# speedup vs baseline: 1.1923x; 1.0193x over previous
"""Biaffine label attention kernel for 8 trn2 NeuronCores.

out[b, l, i, j] = (head[b] @ W_head.T)[i, l] + (dep[b] @ W_dep.T)[j, l] + bias[l]

with head/dep: [8, 512, 512] f32, label_W: [64, 1024], label_b: [64],
out: [8, 64, 512, 512] f32 (512 MB).

Sharding: data-parallel over batch; core b computes the contiguous 64 MB
slice out[b].  The kernel is output-write bound (~419 GB/s per core observed),
so the device program keeps the output DMAs maximally efficient (2 MB
dma_starts with 8 KB contiguous DRAM runs) while TensorE / ScalarE / VectorE
generate tiles well under the DMA rate:

  - Inputs arrive as 4 packed arrays (dma_start issue costs ~0.63 us each on
    the sequencer, so many small loads are issue-rate-limited), with all
    matmul operands pre-split on the host into bf16 hi+lo pairs: bf16
    cross-term matmuls accumulating in fp32 PSUM run ~8x faster than fp32
    matmuls on the PE while keeping ~1e-5 accuracy.
  - TensorE: a short HAM warm-up burst, then d' = dep@Wd^T + b and
    h = head@Wh^T (computed [l, i] and flipped into the swizzled [i, l]
    layout with four full-rate strided PE transposes), then one K=2
    ones-matmul per label over the hi/lo split of d' to broadcast d'[l, :]
    across 128 partitions into PSUM.
  - VectorE + ScalarE: 4 per-partition-scalar adds per label, reading the
    broadcast PSUM tile: out_tile[p, c*512 + j] = d'[l, j] + h[l, 4p + c].
  - Output: the first 4 labels ship as 1 MB DMAs (earliest first bytes),
    then one 2 MB HWDGE DMA per label pair (partition p holds rows
    4p..4p+3 -> 8 KB contiguous runs in DRAM).

Measured on 8 axon trn2 cores: ~198-201 us HW exec (the machine's power
throttle state adds ~±15% run-to-run), rel err ~4e-6.  The output roofline
at the observed ~419 GB/s steady DMA rate is ~160 us; the rest is the
~7 us fixed engine preamble, the serial d'/h prologue, and the final DMA
queue drain.
"""

import os
import sys
from contextlib import ExitStack

for _p in ("/opt/trn_rl_repo",):
    if os.path.isdir(_p) and _p not in sys.path:
        sys.path.insert(0, _p)

import numpy as np

import concourse.bass as bass
import concourse.bacc as bacc
import concourse.masks as masks
import concourse.tile as tile
from concourse import mybir
from concourse.bass_utils import run_bass_kernel_spmd

B = 8
S = 512
D = 512
L = 64
KT = D // 128  # contraction tiles
C = S // 128   # i-rows packed per partition
F32 = mybir.dt.float32

_NC_CACHE = None


def _build_nc():
    nc = bacc.Bacc(
        "TRN2", target_bir_lowering=False, debug=False, num_devices=B
    )
    BF16 = mybir.dt.bfloat16
    # Inputs packed into 4 arrays: dma_start issue costs ~0.63 us each on the
    # sequencer, so many small loads are issue-rate-limited (~200 GB/s).
    dep2d = nc.declare_dram_parameter("dep2", [128, 2 * KT * S], BF16, isOutput=False)
    head2d = nc.declare_dram_parameter("head2", [128, 2 * KT * S], BF16, isOutput=False)
    w4d = nc.declare_dram_parameter("w4", [128, 4 * KT * L], BF16, isOutput=False)
    biasv = nc.declare_dram_parameter("biasv", [L, 1], F32, isOutput=False)
    out = nc.declare_dram_parameter("out", [L, S, S], F32, isOutput=True)

    with tile.TileContext(nc) as tc, ExitStack() as ctx:
        const = ctx.enter_context(tc.tile_pool(name="const", bufs=1))
        psum_bc = ctx.enter_context(tc.tile_pool(name="psum_bc", bufs=5, space="PSUM"))
        psum_hd = ctx.enter_context(tc.tile_pool(name="psum_hd", bufs=1, space="PSUM"))
        out_pool = ctx.enter_context(tc.tile_pool(name="outp", bufs=6))

        # All loads share the sync HWDGE ring with the output writes — using
        # the scalar ring for inputs measurably slowed the steady-state
        # output stream (~400 vs ~419 GB/s), so everything stays on one ring.
        # d-path inputs first: the whole kernel is gated on d' being ready.
        w4 = const.tile([128, 4 * KT * L], BF16)
        nc.sync.dma_start(w4[:], w4d[:, :])
        bcol = const.tile([L, 1], F32)
        nc.sync.dma_start(bcol[:], biasv[:, :])
        dep2 = const.tile([128, 2 * KT * S], BF16)
        nc.sync.dma_start(dep2[:], dep2d[:, :])
        head2 = const.tile([128, 2 * KT * S], BF16)
        nc.sync.dma_start(head2[:], head2d[:, :])

        def wslice(idx, kt):  # w4 packs [wdh | wdl | whh | whl], KT*L each
            base = idx * KT * L + kt * L
            return w4[:, base : base + L]

        def dslice(hi, kt):  # dep2 packs [hi | lo], KT*S each
            base = (0 if hi else KT * S) + kt * S
            return dep2[:, base : base + S]

        def hslice(hi, kt):
            base = (0 if hi else KT * S) + kt * S
            return head2[:, base : base + S]

        ones2 = const.tile([2, 128], BF16)
        nc.vector.memset(ones2[:], 1.0)
        wtile = const.tile([2, S], BF16)
        nc.vector.memset(wtile[:], 0.0)
        ident = const.tile([L, L], F32)
        masks.make_identity(nc, ident[:])

        # PE HAM warm-up: ~4 us of throwaway matmuls while the inputs load,
        # so the real prologue matmuls run at 2.4 GHz instead of 1.2.
        for _ in range(8):
            wp = psum_bc.tile([128, S], F32, tag="bcp")
            nc.tensor.matmul(wp[:], ones2[:], wtile[:], start=True, stop=True)

        # d'[l, j] = sum_d dep[j, d] * W_dep[l, d] + b[l]   (l on partitions)
        # dep ~ dh + dl, wd ~ wh_ + wl_ in bf16; the three large cross terms
        # accumulate in fp32 PSUM at full PE rate (fp32 matmul is ~8x slower;
        # the dropped lo*lo term is ~2^-18 relative).
        dps = psum_hd.tile([L, S], F32)
        n_terms = 3 * KT
        ti = 0
        for kt in range(KT):
            for wi, dh in ((0, 1), (0, 0), (1, 1)):
                nc.tensor.matmul(
                    dps[:],
                    wslice(wi, kt),
                    dslice(dh, kt),
                    start=(ti == 0),
                    stop=(ti == n_terms - 1),
                )
                ti += 1
        d_sb = const.tile([L, S], F32)
        nc.scalar.add(d_sb[:], dps[:], bcol[:])

        # Split d' into bf16 hi + lo so the per-label broadcast matmul runs at
        # full PE rate (fp32 matmul is ~8x slower); hi+lo recovers ~fp32
        # precision since PSUM accumulates in fp32.
        d_hi = const.tile([L, S], BF16)
        nc.vector.tensor_copy(d_hi[:], d_sb[:])
        d_hi32 = const.tile([L, S], F32)
        nc.scalar.copy(d_hi32[:], d_hi[:])
        d_lo = const.tile([L, S], BF16)
        nc.vector.tensor_sub(d_lo[:], d_sb[:], d_hi32[:])

        # Flatten [L, S] (l on partitions) -> rows of one [2, L*S] tile so the
        # broadcast rhs [2, S] can be sliced at partition base 0 for any l
        # (engine APs may only start at partition 0/32/64).
        d2 = const.tile([2, L * S], BF16)
        d2v = d2[:].rearrange("p (l j) -> p l j", l=L)
        nc.sync.dma_start(d2v[0:1, :, :], d_hi[:])
        nc.sync.dma_start(d2v[1:2, :, :], d_lo[:])

        # h[l, i] = sum_d head[i, d] * W_head[l, d], as bf16 hi/lo cross terms
        # (full PE rate; fp32 N=64 matmuls measured 2 HW passes = ~13 us and
        # gated the whole pipeline).  Result lands [l, i]; four strided PE
        # transposes then produce the swizzled [i, l] layout the adds need:
        # h_sw[p, c*L + l] = h[l, 4p + c].
        hps_li = psum_hd.tile([L, S], F32)
        ti = 0
        for kt in range(KT):
            for wi, hh in ((2, 1), (2, 0), (3, 1)):
                nc.tensor.matmul(
                    hps_li[:],
                    wslice(wi, kt),
                    hslice(hh, kt),
                    start=(ti == 0),
                    stop=(ti == 3 * KT - 1),
                )
                ti += 1
        h_li = const.tile([L, S], F32)
        nc.scalar.copy(h_li[:], hps_li[:])
        h_li_str = h_li[:].rearrange("l (m c) -> l c m", c=C)
        hps_sw = psum_hd.tile([128, C * L], F32)
        for c in range(C):
            nc.tensor.transpose(
                hps_sw[:, c * L : (c + 1) * L], h_li_str[:, c, :], ident[:]
            )
        h_sw = const.tile([128, C * L], F32)
        nc.scalar.copy(h_sw[:], hps_sw[:])

        # out[l, 4p + c, j] = d'[l, j] + h_sw[p, c*L + l]
        # The first few labels ship as single 1 MB DMAs so output bytes start
        # flowing as early as possible; the rest as 2 MB label-pair DMAs.
        N_WARM = 4
        out_r1 = out[:, :, :].rearrange("l (p c) j -> l p (c j)", c=C)
        out_r = out[:, :, :].rearrange("(lp m) (p c) j -> lp p m (c j)", m=2, c=C)

        def emit_label(l, ot, fbase):
            """Broadcast d'[l] and add h columns into ot[:, fbase:fbase+C*S]."""
            bcp = psum_bc.tile([128, S], F32)
            nc.tensor.matmul(bcp[:], ones2[:], d2v[:, l, :], start=True, stop=True)
            for c in range(C):
                scalar = h_sw[:, c * L + l : c * L + l + 1]
                dst = ot[:, fbase + c * S : fbase + (c + 1) * S]
                if c < 2:
                    nc.vector.tensor_scalar_add(dst, bcp[:], scalar)
                else:
                    nc.scalar.add(dst, bcp[:], scalar)

        warm_pool = ctx.enter_context(tc.tile_pool(name="warm", bufs=2))
        for l in range(N_WARM):
            ot = warm_pool.tile([128, C * S], F32)
            emit_label(l, ot, 0)
            nc.sync.dma_start(out_r1[l], ot[:])
        for lp in range(N_WARM // 2, L // 2):
            ot = out_pool.tile([128, 2 * C * S], F32)
            for m in range(2):
                emit_label(2 * lp + m, ot, m * C * S)
            nc.sync.dma_start(out_r[lp], ot[:])
    nc.compile()
    return nc


def _row_tile(a):
    """[D, F] -> [128, KT*F]: row d = kt*128 + p lands at [p, kt*F : (kt+1)*F]."""
    d, f = a.shape
    kt = d // 128
    return np.ascontiguousarray(
        a.reshape(kt, 128, f).transpose(1, 0, 2).reshape(128, kt * f)
    )


def _hi_lo(a):
    """f32 array -> (bf16 hi, bf16 lo) with a ~ hi + lo."""
    import ml_dtypes

    hi = a.astype(ml_dtypes.bfloat16)
    lo = (a - hi.astype(np.float32)).astype(ml_dtypes.bfloat16)
    return np.ascontiguousarray(hi), np.ascontiguousarray(lo)


def _prep_inputs(head, dep, label_W, label_b):
    head = np.asarray(head, dtype=np.float32)
    dep = np.asarray(dep, dtype=np.float32)
    label_W = np.asarray(label_W, dtype=np.float32)
    label_b = np.asarray(label_b, dtype=np.float32)

    whh, whl = _hi_lo(_row_tile(np.ascontiguousarray(label_W[:, :D].T)))
    wdh, wdl = _hi_lo(_row_tile(np.ascontiguousarray(label_W[:, D:].T)))
    w4 = np.ascontiguousarray(np.concatenate([wdh, wdl, whh, whl], axis=1))
    bias = np.ascontiguousarray(label_b.reshape(L, 1))

    in_maps = []
    for b in range(B):
        hth, htl = _hi_lo(_row_tile(np.ascontiguousarray(head[b].T)))
        dth, dtl = _hi_lo(_row_tile(np.ascontiguousarray(dep[b].T)))
        in_maps.append(
            {
                "head2": np.ascontiguousarray(np.concatenate([hth, htl], axis=1)),
                "dep2": np.ascontiguousarray(np.concatenate([dth, dtl], axis=1)),
                "w4": w4,
                "biasv": bias,
            }
        )
    return in_maps


def _run(head, dep, label_W, label_b, trace=False, **trace_kwargs):
    global _NC_CACHE
    if _NC_CACHE is None:
        _NC_CACHE = _build_nc()
    in_maps = _prep_inputs(head, dep, label_W, label_b)
    res = run_bass_kernel_spmd(
        _NC_CACHE, in_maps, list(range(B)), trace=trace, **trace_kwargs
    )
    out = np.stack([res.results[i]["out"] for i in range(B)])
    return out, res


def kernel(head, dep, label_W, label_b):
    out, _ = _run(head, dep, label_W, label_b, trace=False)
    return out
